# revision 28
# baseline (speedup 1.0000x reference)
"""ComplEx scoring kernel for 8 Trainium2 NeuronCores.

Math: score[b, e] = Re(<h_b * r_b, conj(ent_e)>) with h = ent_emb[triples[:,0]],
r = rel_emb[triples[:,1]].  Writing ans_b = concat(re_h*re_r - im_h*im_r,
re_h*im_r + im_h*re_r) (shape [B, 512]), the score is exactly
score = ans @ ent_emb.T  — one [1024, 512] x [512, 200000] GEMM.

Strategy (vocab/tensor parallel on the entity axis, 25000 entities/core,
padded to 25008 columns):
  - host: tiny gather + complex multiply -> ans  (microseconds)
  - the GEMM is TensorE-bound (26.3 GFLOP/core vs 78.6 TF/s bf16), so the
    entity axis is split into a bf16 part (34.5 tiles of 512) and an
    fp8-e4m3 DoubleRow part (7344 cols) running the PE at 2 MACs/cell/cy.
  - fp8 noise is relative per column, so each column's squared error is
    proportional to its score energy ~ ||ent_row||^2.  Assigning the
    smallest-norm entities of each shard to the fp8 region (instead of an
    arbitrary 26%) cuts the fp8 error ~8%, which funds a 29.3% fp8
    fraction at the same global rel err (~1.96e-2 < 2e-2).  The host
    unscrambles the column permutation during assemble.
  - per core: score_bf16[1024, 17664] + score_fp8[1024, 7344], both f16.
    PE is pre-warmed with dummy matmuls so the HAM clock gate opens during
    the preamble/first DMAs.  Group 0 is small (2 tiles) so the warm
    loop's consumption stays inside what the still-ramping input queues
    deliver.  Inner loops are
    tile-outer so each PSUM bank is freed ~5us before pool reuse.  The
    fp8 section runs second to last; a final 768-column bf16 group
    absorbs its write backlog so the kernel tail is one small DMA.
  - host: per-region unscale + column scatter back to entity order.
"""

import numpy as np
import ml_dtypes

NCORES = 8
NUM_ENT = 200000
EMB = 512
B = 1024
SHARD = NUM_ENT // NCORES      # 25000 entities per core
NTILE = 512                    # matmul moving free dim == one PSUM bank
NB_FULL = 34                   # full bf16 512-tiles per core
NB_PART = 256                  # trailing partial bf16 tile width
NB = NB_FULL * NTILE + NB_PART # 17664 bf16 columns (all real)
NF_FULL = 14                   # full fp8 512-tiles per core
NF_PART = 176                  # trailing partial fp8 tile width
NF = NF_FULL * NTILE + NF_PART # 7344 fp8 columns (7336 real + 8 pad)
SHARD_PAD = NB + NF            # 25088
NF_REAL = SHARD - NB           # 7336 real entities in the fp8 region
GROUPS = [2, 5, 7, 7, 7, 5]    # full bf16 tile groups (DMA/reuse granularity)
NGRP = 33 * NTILE              # columns covered by GROUPS
NFIN = NB - NGRP               # 768: final group = 1 full tile + partial
GN_FULL = 7 * NTILE            # 3584
KCH = EMB // 128               # 4 contraction chunks
MCH = B // 128                 # 8 batch chunks
WARMUP_MMS = 8

_NC = None
_SCALES = {}

# bf16-path score values are ~1e-5 — subnormal in fp16.  Pre-scaling ans by
# 2**16 on the host puts the device-side scores in fp16's normal range; the
# host unscales.  The fp8 path has its own scales (s_a, s_e) chosen at prep
# time so quantized inputs sit in e4m3's range and scores fit fp16.
OUT_SCALE = 2.0 ** 16


def _build_nc():
    import concourse.bacc as bacc
    import concourse.bass as bass
    import concourse.tile as tile
    from concourse import mybir

    ts, ds = bass.ts, bass.ds
    bf16 = mybir.dt.bfloat16
    f16 = mybir.dt.float16
    f8 = mybir.dt.float8e4
    f32 = mybir.dt.float32
    DR = mybir.MatmulPerfMode.DoubleRow

    nc = bacc.Bacc("TRN2", target_bir_lowering=False, debug=False)
    ansT = nc.dram_tensor("ansT", [EMB, B], bf16, kind="ExternalInput")
    ans8 = nc.dram_tensor("ans8", [128, KCH, B], f8, kind="ExternalInput")
    entT = nc.dram_tensor("entT", [EMB, NGRP], bf16, kind="ExternalInput")
    entP = nc.dram_tensor("entP", [128, KCH, NFIN], bf16,
                          kind="ExternalInput")
    ent8 = nc.dram_tensor("ent8", [128, KCH, NF], f8, kind="ExternalInput")
    score = nc.dram_tensor("score", [B, SHARD_PAD], f16, kind="ExternalOutput")

    with tile.TileContext(nc) as tc:
        with tc.tile_pool(name="entp", bufs=2 * KCH) as ent_pool, \
             tc.tile_pool(name="outp", bufs=5) as out_pool, \
             tc.tile_pool(name="out8p", bufs=3) as out8_pool, \
             tc.tile_pool(name="ps", bufs=8, space="PSUM") as psum_pool:

            _frees = []
            ansT_sb, _f = tc.tile([128, KCH, B], bf16, name="ansT_sb")
            _frees.append(_f)
            ans8_sb, _f = tc.tile([128, KCH, B], f8, name="ans8_sb")
            _frees.append(_f)
            entP_sb, _f = tc.tile([128, KCH, NFIN], bf16, name="entP_sb")
            _frees.append(_f)
            ent8_sb, _f = tc.tile([128, KCH, NF], f8, name="ent8_sb")
            _frees.append(_f)
            wup, _f = tc.tile([128, 640], bf16, name="wup")
            _frees.append(_f)

            # PE pre-warm: zero matmuls so the HAM clock gate opens during
            # the preamble/DMA wait; real matmuls then run at 2.4 GHz.
            nc.gpsimd.memset(wup[:], 0)
            wps = psum_pool.tile([128, NTILE], f32, name="pst")
            for i in range(WARMUP_MMS):
                nc.tensor.matmul(wps[:], wup[:, ds(0, 128)],
                                 wup[:, ds(128, 512)],
                                 start=(i == 0), stop=(i == WARMUP_MMS - 1))

            # startup: the first real matmul needs ansT[k0] + the group-0
            # k0 ent slab.  Those two DMAs are issued in parallel, one on
            # the SP ring (idle until the first score write) and one on
            # the ACT ring, so the dependency lands ~1us earlier than a
            # serial issue on one ring.  Group 0 is deliberately small
            # (2 tiles): the warm loop's first k-cycles then consume input
            # at ~290 GB/s, inside what HBM can deliver while the queues
            # are still ramping.
            ent_sb0 = [ent_pool.tile([128, GN_FULL], bf16, name="ent_sb")
                       for _ in range(KCH)]
            gn0 = GROUPS[0] * NTILE
            # both first loads ride the ACT ring serially: the SP ring's
            # first-DMA service latency is erratic (up to ~2us late), and a
            # late ansT stalls the whole warm loop
            nc.scalar.dma_start(ansT_sb[:, 0], ansT[ts(0, 128), :])
            nc.scalar.dma_start(ent_sb0[0][:, ds(0, gn0)],
                                entT[ts(0, 128), ds(0, gn0)])
            for k in range(1, KCH):
                nc.scalar.dma_start(ansT_sb[:, k], ansT[ts(k, 128), :])
                nc.scalar.dma_start(ent_sb0[k][:, ds(0, gn0)],
                                    entT[ts(k, 128), ds(0, gn0)])

            # inputs ride the ACT HWDGE ring (nc.scalar) — it keeps
            # prefetches from queueing behind score-output DMAs on SP.
            def load_group(g, gcol):
                # one tile per k-chunk so a matmul only waits for its own DMA
                gn = GROUPS[g] * NTILE
                tiles = []
                for k in range(KCH):
                    t = ent_pool.tile([128, GN_FULL], bf16, name="ent_sb")
                    nc.scalar.dma_start(t[:, ds(0, gn)],
                                        entT[ts(k, 128), ds(gcol, gn)])
                    tiles.append(t)
                return tiles

            # gpsimd (Pool) cannot read PSUM on TRN2 — copyback on DVE + Act
            copy_engines = [nc.vector, nc.scalar]
            ci = 0

            def copyback(dst, ps):
                nonlocal ci
                eng = copy_engines[ci % len(copy_engines)]
                ci += 1
                if eng is nc.scalar:
                    eng.copy(dst, ps)
                else:
                    eng.tensor_copy(out=dst, in_=ps)

            ent_tiles = {0: ent_sb0}
            gcols = np.cumsum([0] + [gs * NTILE for gs in GROUPS]).tolist()
            COLP = gcols[-1]               # partial bf16 tile column base
            COL8 = NB                      # fp8 region column base

            def load_fp8():
                nc.scalar.dma_start(ans8_sb[:], ans8[:, :, :])
                nc.scalar.dma_start(ent8_sb[:, ds(0, 2)], ent8[:, ds(0, 2), :])
                nc.scalar.dma_start(ent8_sb[:, ds(2, 2)], ent8[:, ds(2, 2), :])

            def load_part():
                nc.scalar.dma_start(entP_sb[:], entP[:, :, :])

            def bf16_group(g, warm=False):
                gsz = GROUPS[g]
                gn = gsz * NTILE
                col = gcols[g]
                ent_sb = ent_tiles.pop(g)

                if warm:
                    # warm-up: k-outer with m0..m3 interleaved (4*gsz = 8
                    # psum banks) so each k ent slab feeds 16 matmuls
                    # (~3.5us) — each k-cycle only consumes a 256 KB slab,
                    # so the ramping input queue keeps the PE fed
                    WM = 4
                    outs = [out_pool.tile([128, GN_FULL], f16, name="out_sb")
                            for _ in range(WM)]
                    pss0 = [[psum_pool.tile([128, NTILE], f32, name="pst")
                             for _ in range(gsz)] for _ in range(WM)]
                    for k in range(KCH):
                        for m in range(WM):
                            lhsT = ansT_sb[:, k, ts(m, 128)]
                            for t in range(gsz):
                                nc.tensor.matmul(
                                    pss0[m][t][:], lhsT,
                                    ent_sb[k][:, ts(t, NTILE)],
                                    start=(k == 0), stop=(k == KCH - 1))
                                if k == KCH - 1:
                                    copyback(outs[m][:, ts(t, NTILE)],
                                             pss0[m][t][:])
                    for m in range(WM):
                        nc.sync.dma_start(score[ts(m, 128), ds(col, gn)],
                                          outs[m][:, ds(0, gn)])
                    ms = range(WM, MCH)
                else:
                    ms = range(MCH)

                for m in ms:
                    pss = [psum_pool.tile([128, NTILE], f32, name="pst")
                           for _ in range(gsz)]
                    out_sb = out_pool.tile([128, GN_FULL], f16, name="out_sb")
                    # tile outer: each tile's copyback fires right after its
                    # 4 accumulating matmuls, so PSUM banks are freed ~5us
                    # before the pool reuses them (k-outer bunched all the
                    # copies at the m-step's end and start-MMs stalled on
                    # bank recycling every ~49 MMs)
                    for t in range(gsz):
                        for k in range(KCH):
                            nc.tensor.matmul(
                                pss[t][:], ansT_sb[:, k, ts(m, 128)],
                                ent_sb[k][:, ts(t, NTILE)],
                                start=(k == 0), stop=(k == KCH - 1))
                        copyback(out_sb[:, ts(t, NTILE)], pss[t][:])
                    if gsz >= 4:
                        # two half-width output DMAs so the drain starts as
                        # soon as the first copies land
                        h0 = (gsz // 2 + 1) * NTILE
                        nc.sync.dma_start(score[ts(m, 128), ds(col, h0)],
                                          out_sb[:, ds(0, h0)])
                        nc.sync.dma_start(
                            score[ts(m, 128), ds(col + h0, gn - h0)],
                            out_sb[:, ds(h0, gn - h0)])
                    else:
                        nc.sync.dma_start(score[ts(m, 128), ds(col, gn)],
                                          out_sb[:, ds(0, gn)])

            def fp8_section():
                # fp8 DoubleRow: K=512 as 2 matmuls of 256 (2 fp8/cell).
                # Runs second to last: its score writes come at 2x the bf16
                # rate (~300 GB/s); the SP ring absorbs them at ~340 GB/s
                # once the input loads are done, and the final small bf16
                # group drains the residue.
                for m in range(MCH):
                    out_sb = out8_pool.tile([128, NF], f16, name="out8_sb")
                    for t in range(NF_FULL):
                        ps = psum_pool.tile([128, NTILE], f32, name="pst")
                        for j in range(2):
                            nc.tensor.matmul(
                                ps[:],
                                ans8_sb[:, ds(2 * j, 2), ts(m, 128)],
                                ent8_sb[:, ds(2 * j, 2), ds(t * NTILE, NTILE)],
                                start=(j == 0), stop=(j == 1),
                                perf_mode=DR)
                        copyback(out_sb[:, ts(t, NTILE)], ps[:])
                    # trailing partial fp8 tile
                    ps = psum_pool.tile([128, NTILE], f32, name="pst")
                    for j in range(2):
                        nc.tensor.matmul(
                            ps[:, ds(0, NF_PART)],
                            ans8_sb[:, ds(2 * j, 2), ts(m, 128)],
                            ent8_sb[:, ds(2 * j, 2),
                                    ds(NF_FULL * NTILE, NF_PART)],
                            start=(j == 0), stop=(j == 1),
                            perf_mode=DR)
                    copyback(out_sb[:, ds(NF_FULL * NTILE, NF_PART)],
                             ps[:, ds(0, NF_PART)])
                    h0 = 7 * NTILE
                    nc.sync.dma_start(score[ts(m, 128), ds(COL8, h0)],
                                      out_sb[:, ds(0, h0)])
                    nc.sync.dma_start(score[ts(m, 128), ds(COL8 + h0, NF - h0)],
                                      out_sb[:, ds(h0, NF - h0)])

            def final_group():
                # final 768-column bf16 group (1 full tile + the 256-col
                # partial): ~11us of PE time that absorbs the fp8 burst's
                # copyback/DMA drain.  Full-tile writes go on SP; the
                # partial's copy+write run on ACT/its own issue so the last
                # m-step's chain is one small 64KB DMA.
                for m in range(MCH):
                    psf = psum_pool.tile([128, NTILE], f32, name="pst")
                    psp = psum_pool.tile([128, NTILE], f32, name="pst")
                    out_sb = out_pool.tile([128, GN_FULL], f16, name="out_sb")
                    for k in range(KCH):
                        lhsT = ansT_sb[:, k, ts(m, 128)]
                        nc.tensor.matmul(psf[:], lhsT,
                                         entP_sb[:, k, ds(0, NTILE)],
                                         start=(k == 0), stop=(k == KCH - 1))
                    for k in range(KCH):
                        lhsT = ansT_sb[:, k, ts(m, 128)]
                        nc.tensor.matmul(psp[:, ds(0, NB_PART)], lhsT,
                                         entP_sb[:, k, ds(NTILE, NB_PART)],
                                         start=(k == 0), stop=(k == KCH - 1))
                    nc.vector.tensor_copy(out=out_sb[:, ds(0, NTILE)],
                                          in_=psf[:])
                    nc.sync.dma_start(score[ts(m, 128), ds(COLP, NTILE)],
                                      out_sb[:, ds(0, NTILE)])
                    nc.scalar.copy(out_sb[:, ds(NTILE, NB_PART)],
                                   psp[:, ds(0, NB_PART)])
                    nc.scalar.dma_start(
                        score[ts(m, 128), ds(COLP + NTILE, NB_PART)],
                        out_sb[:, ds(NTILE, NB_PART)])

            # process order: b0..b5, fp8, final — each section's inputs are
            # issued at least one section ahead on the ACT ring.
            ent_tiles[1] = load_group(1, gcols[1])
            bf16_group(0, warm=True)
            ent_tiles[2] = load_group(2, gcols[2])
            bf16_group(1)
            ent_tiles[3] = load_group(3, gcols[3])
            bf16_group(2)
            ent_tiles[4] = load_group(4, gcols[4])
            bf16_group(3)
            load_fp8()
            bf16_group(4)
            ent_tiles[5] = load_group(5, gcols[5])
            load_part()
            bf16_group(5)
            fp8_section()
            final_group()
            for _f in reversed(_frees):
                _f()
    nc.compile()
    return nc


def _get_nc():
    global _NC
    if _NC is None:
        _NC = _build_nc()
    return _NC


def _pmap(fn, n):
    from concurrent.futures import ThreadPoolExecutor
    with ThreadPoolExecutor(max_workers=n) as ex:
        list(ex.map(fn, range(n)))


def _to_f8_chunks(mat_t, ncols):
    """[EMB, ncols] f32 (already scaled) -> [128, KCH, ncols] e4m3 bytes."""
    q = mat_t.astype(ml_dtypes.float8_e4m3fn)
    return np.ascontiguousarray(q.reshape(KCH, 128, ncols).transpose(1, 0, 2))


def prepare_in_maps(triples, ent_emb, rel_emb):
    triples = np.asarray(triples)
    ent_emb = np.asarray(ent_emb, dtype=np.float32)
    rel_emb = np.asarray(rel_emb, dtype=np.float32)

    d = EMB // 2
    h = ent_emb[triples[:, 0].astype(np.int64)]
    r = rel_emb[triples[:, 1].astype(np.int64)]
    re_h, im_h = h[:, :d], h[:, d:]
    re_r, im_r = r[:, :d], r[:, d:]
    ans = np.empty((B, EMB), np.float32)
    ans[:, :d] = re_h * re_r - im_h * im_r
    ans[:, d:] = re_h * im_r + im_h * re_r

    ansT_bf = np.ascontiguousarray(ans.T * np.float32(OUT_SCALE)).astype(
        ml_dtypes.bfloat16)

    # fp8 noise per column scales with the column's score energy, i.e. with
    # ||ent_row||^2 — so the smallest-norm entities of each shard go to the
    # fp8 region.  The host scatters columns back during assemble.
    norms = (ent_emb * ent_emb).sum(1)
    bf_idx = np.empty((NCORES, NB), np.int64)
    f8_idx = np.empty((NCORES, NF_REAL), np.int64)
    for c in range(NCORES):
        sl = norms[c * SHARD:(c + 1) * SHARD]
        order = np.argpartition(sl, NF_REAL)
        f8_idx[c] = np.sort(order[:NF_REAL])
        bf_idx[c] = np.sort(order[NF_REAL:])
    _SCALES["bf_idx"] = bf_idx
    _SCALES["f8_idx"] = f8_idx

    # fp8 scales: map absmax to ~120 (TRN e4m3 max 240), then cap the product
    # so the Cauchy-Schwarz bound on device-side scores stays inside fp16
    f8_rows = np.concatenate([
        ent_emb[c * SHARD + f8_idx[c]] for c in range(NCORES)])
    amax_a = float(np.abs(ans).max())
    amax_e = float(np.abs(f8_rows).max())
    s_a = 120.0 / amax_a
    s_e = 120.0 / amax_e
    cs = float(np.sqrt((ans * ans).sum(1).max()) *
               np.sqrt((f8_rows * f8_rows).sum(1).max()))
    cap = 58000.0 / cs
    if s_a * s_e > cap:
        s_a = cap / s_e
    _SCALES["fp8_inv"] = 1.0 / (s_a * s_e)

    ans8 = _to_f8_chunks(np.ascontiguousarray(ans.T) * np.float32(s_a), B)

    ent_bf = np.empty((NCORES, EMB, NGRP), dtype=ml_dtypes.bfloat16)
    ent_pp = np.empty((NCORES, 128, KCH, NFIN), dtype=ml_dtypes.bfloat16)
    ent8s = np.empty((NCORES, 128, KCH, NF), dtype=ml_dtypes.float8_e4m3fn)

    def _core(c):
        rows = ent_emb[c * SHARD:(c + 1) * SHARD]
        bfT = rows[bf_idx[c]].T                      # [EMB, NB]
        ent_bf[c] = bfT[:, :NGRP]
        pp = np.ascontiguousarray(bfT[:, NGRP:])
        ent_pp[c] = pp.reshape(KCH, 128, NFIN).transpose(1, 0, 2)
        blk = np.zeros((EMB, NF), np.float32)
        blk[:, :NF_REAL] = rows[f8_idx[c]].T * np.float32(s_e)
        ent8s[c] = _to_f8_chunks(blk, NF)

    _pmap(_core, NCORES)
    return [{"ansT": ansT_bf, "ans8": ans8, "entT": ent_bf[c],
             "entP": ent_pp[c], "ent8": ent8s[c]} for c in range(NCORES)]


def run_raw(in_maps, trace=False):
    from concourse import bass_utils
    return bass_utils.run_bass_kernel_spmd(
        _get_nc(), in_maps, core_ids=list(range(NCORES)), trace=trace
    )


def assemble(results):
    out = np.empty((B, NUM_ENT), np.float32)
    inv16 = np.float32(1.0 / OUT_SCALE)
    inv8 = np.float32(_SCALES["fp8_inv"])
    bf_idx = _SCALES["bf_idx"]
    f8_idx = _SCALES["f8_idx"]

    def _one(c):
        sh = results[c]["score"]
        bf = sh[:, :NB].astype(np.float32)
        bf *= inv16
        f8 = sh[:, NB:NB + NF_REAL].astype(np.float32)
        f8 *= inv8
        base = c * SHARD
        out[:, base + bf_idx[c]] = bf
        out[:, base + f8_idx[c]] = f8

    _pmap(_one, NCORES)
    return out


def kernel(triples, ent_emb, rel_emb):
    in_maps = prepare_in_maps(triples, ent_emb, rel_emb)
    res = run_raw(in_maps)
    return assemble(res.results)
